# revision 5
# baseline (speedup 1.0000x reference)
"""GIN encoder (3-layer, N=50000, E=800000, D=128) on 8 trn2 NeuronCores.

Strategy (graph/data parallel, hardcoded):
  - Nodes padded to 50176 = 8 cores x 6272 (49 windows x 128). Each core
    owns a contiguous block of 6272 destination nodes.
  - Edges partitioned by destination core; cells keyed by (dst window-pair,
    src-part) where src-part A = first 3072 node-block rows of the source's
    core, B = last 3200. Chunks of 128 edges, padded to a uniform count C
    per cell so all cores run one SPMD program.
  - Per layer: dma_gather z[src] rows (fp16, 256B rows, 1024 idxs/gather =
    the SWDGE ring cap); segment-sum via 0/1-indicator matmuls over 256-slot
    window pairs (fp16 -> fp32 PSUM, one PSUM bank per pair so accumulation
    groups never share a zero region); fused GIN MLP (fp16 weights, fp32
    bias+relu on ACT); an extra transposed per-window matmul makes
    node-major fp16 z for the halo exchange.
  - Halo exchange: two AllGathers per layer (A-part after the first 24
    windows finish, B-part at layer end); the ~500us collective latency
    hides under the next chunk of gather work.
  - dma_gather idxs are int16; the A/B tables are 24576/25600 rows, so
    indices stay in range without further splitting.
"""

import numpy as np

N = 50000
E = 800000
D = 128
L = 3
NCORES = 8
NPAD = 50176             # 8 * 6272
PER_CORE = 6272          # 49 * 128
W = 49                   # windows per core
NPAIR = 25               # window pairs (last is a lone window)
WA = 24                  # windows in part A of the halo exchange
NA = WA * 128            # 3072
NB = PER_CORE - NA       # 3200
GW = 8                   # windows per processing group (4 pair-banks)
GROUPS = [list(range(g, min(g + GW, W))) for g in range(0, W, GW)]
AGA_GROUP = 2            # after this group index, windows 0..23 are done
PAD_SLOT = 300.0         # outside [0, 256) -> zero indicator column
MAXG = 1024              # max idxs per dma_gather (SWDGE ring cap)
C_LONE = None            # chunk count of the lone last pair (set at prep)


def _pair_chunks(C, npairs_in_group):
    return npairs_in_group * C


def _prepare_edges(edge_index):
    """Partition/pad edges -> per-core idx (int16) + dstslot (fp16) arrays.

    Cell = (dst window-pair, src part); dst slot in [0, 256). Flat chunk
    order: (group, part, pair, chunk). Returns (C, Cl, idx_arrs, dst_arrs).
    """
    src = np.asarray(edge_index[0], dtype=np.int64)
    dst = np.asarray(edge_index[1], dtype=np.int64)

    core = dst // PER_CORE
    local = dst % PER_CORE
    pair = local // 256                      # 0..24 (pair 24 = lone window)
    slot = local % 256

    s_core = src // PER_CORE
    s_ln = src % PER_CORE
    part = (s_ln >= NA).astype(np.int64)
    idxval = np.where(part == 0, s_core * NA + s_ln,
                      s_core * NB + (s_ln - NA))

    cell = (core * NPAIR + pair) * 2 + part
    n_cells = NCORES * NPAIR * 2
    counts = np.bincount(cell, minlength=n_cells)
    cc = counts.reshape(NCORES, NPAIR, 2)
    C = int(np.ceil(cc[:, :NPAIR - 1].max() / 128))
    Cl = int(np.ceil(cc[:, NPAIR - 1].max() / 128))

    order = np.lexsort((idxval, cell))
    cell_s = cell[order]
    idx_s = idxval[order]
    slot_s = slot[order]

    caps = np.where(np.arange(n_cells) // 2 % NPAIR == NPAIR - 1,
                    Cl * 128, C * 128)
    cell_starts = np.zeros(n_cells + 1, np.int64)
    np.cumsum(counts, out=cell_starts[1:])
    pos = np.arange(E) - cell_starts[cell_s]
    cap_starts = np.zeros(n_cells + 1, np.int64)
    np.cumsum(caps, out=cap_starts[1:])
    flat = cap_starts[cell_s] + pos

    tot = int(cap_starts[-1])
    idx_flat = np.zeros(tot, np.int64)
    slot_flat = np.full(tot, PAD_SLOT, np.float64)
    idx_flat[flat] = idx_s
    slot_flat[flat] = slot_s

    # per (core, pair, part) ragged blocks -> flat (group, part, pair, chunk)
    idx_arrs, dst_arrs = [], []
    for c in range(NCORES):
        icols, dcols = [], []
        for wins in GROUPS:
            pairs = sorted({w // 2 for w in wins})
            for p in (0, 1):
                blocks_i, blocks_s = [], []
                for pr in pairs:
                    cid = (c * NPAIR + pr) * 2 + p
                    s0, s1 = cap_starts[cid], cap_starts[cid + 1]
                    blocks_i.append(idx_flat[s0:s1])
                    blocks_s.append(slot_flat[s0:s1])
                blk_i = np.concatenate(blocks_i)
                blk_s = np.concatenate(blocks_s).reshape(-1, 128).T
                wrapped = blk_i.reshape(-1, 16).T
                icols.append(np.tile(wrapped, (8, 1)))
                dcols.append(blk_s)
        idx_arrs.append(np.concatenate(icols, axis=1).astype(np.int16))
        dst_arrs.append(np.concatenate(dcols, axis=1).astype(np.float16))
    return C, Cl, idx_arrs, dst_arrs


def _gather_sizes(nch):
    per = MAXG // 128
    return [min(per, nch - k) for k in range(0, nch, per)]


def _build_program(C, Cl, n_devices=NCORES, collectives=True, taps=False):
    import concourse.bacc as bacc
    import concourse.tile as tile
    import concourse.mybir as mybir
    from contextlib import ExitStack

    f32 = mybir.dt.float32
    f16 = mybir.dt.float16
    i16 = mybir.dt.int16
    Relu = mybir.ActivationFunctionType.Relu

    nc = bacc.Bacc("TRN2", debug=False, enable_asserts=False,
                   target_bir_lowering=False, num_devices=n_devices)

    TOTCH = (NPAIR - 1) * 2 * C + 2 * Cl
    TOTIC = TOTCH * 8

    xA_t = nc.dram_tensor("xA", [NCORES * NA, D], f16, kind="ExternalInput")
    xB_t = nc.dram_tensor("xB", [NCORES * NB, D], f16, kind="ExternalInput")
    xT32_t = nc.dram_tensor("xT32", [D, PER_CORE], f32, kind="ExternalInput")
    w1_t = nc.dram_tensor("w1", [D, L * D], f16, kind="ExternalInput")
    w2_t = nc.dram_tensor("w2", [D, L * D], f16, kind="ExternalInput")
    b1_t = nc.dram_tensor("b1", [D, L], f32, kind="ExternalInput")
    b2_t = nc.dram_tensor("b2", [D, L], f32, kind="ExternalInput")
    b2m_t = nc.dram_tensor("b2mat", [D, L * D], f32, kind="ExternalInput")
    iota_t = nc.dram_tensor("iota", [D, (MAXG // 128) * 256], f16,
                            kind="ExternalInput")
    idx_t = nc.dram_tensor("idx", [128, TOTIC], i16, kind="ExternalInput")
    dst_t = nc.dram_tensor("dsts", [128, TOTCH], f16, kind="ExternalInput")
    zout_t = nc.dram_tensor("zout", [D, PER_CORE], f32, kind="ExternalOutput")
    if taps:
        agg_o = nc.dram_tensor("agg_o", [128, GW * 128], f32,
                               kind="ExternalOutput")

    rg = [list(range(NCORES))]

    with tile.TileContext(nc) as tc, ExitStack() as ctx:
        const = ctx.enter_context(tc.tile_pool(name="const", bufs=1))
        ztp = ctx.enter_context(tc.tile_pool(name="zt", bufs=1))
        gp = ctx.enter_context(tc.tile_pool(name="g", bufs=4))
        mp = ctx.enter_context(tc.tile_pool(name="m", bufs=3))
        hp = ctx.enter_context(tc.tile_pool(name="h", bufs=2))
        zbp = ctx.enter_context(tc.tile_pool(name="zb", bufs=2))
        aggp = ctx.enter_context(tc.tile_pool(name="agg", bufs=4, space="PSUM"))
        p1p = ctx.enter_context(tc.tile_pool(name="p1", bufs=2, space="PSUM"))
        p2p = ctx.enter_context(tc.tile_pool(name="p2", bufs=2, space="PSUM"))
        dram = ctx.enter_context(tc.tile_pool(name="dram", bufs=1, space="DRAM"))

        w1s = const.tile([D, L * D], f16)
        w2s = const.tile([D, L * D], f16)
        b1s = const.tile([D, L], f32)
        b2s = const.tile([D, L], f32)
        b2ms = const.tile([D, L * D], f32)
        iotas = const.tile([D, (MAXG // 128) * 256], f16)
        idxs = const.tile([128, TOTIC], i16)
        dsts = const.tile([128, TOTCH], f16)
        for sb, t in ((w1s, w1_t), (w2s, w2_t), (b1s, b1_t), (b2s, b2_t),
                      (b2ms, b2m_t), (iotas, iota_t), (idxs, idx_t),
                      (dsts, dst_t)):
            nc.sync.dma_start(sb[:], t.ap())

        ztA = ztp.tile([D, PER_CORE], f32)
        ztB = ztp.tile([D, PER_CORE], f32)
        z16 = ztp.tile([128, W * 128], f16)
        nc.sync.dma_start(ztA[:], xT32_t.ap())
        z16r = z16.rearrange("p (w d) -> p w d", d=128)

        zblkA = [dram.tile([NA, D], f16, name=f"zblkA{l}", tag=f"zblkA{l}")
                 for l in range(L - 1)]
        zblkB = [dram.tile([NB, D], f16, name=f"zblkB{l}", tag=f"zblkB{l}")
                 for l in range(L - 1)]
        sh = "Shared" if collectives else "Local"
        zshA = [dram.tile([NCORES * NA, D], f16, addr_space=sh,
                          name=f"zshA{l}", tag=f"zshA{l}") for l in range(L - 1)]
        zshB = [dram.tile([NCORES * NB, D], f16, addr_space=sh,
                          name=f"zshB{l}", tag=f"zshB{l}") for l in range(L - 1)]

        def halo(l, blk, shr, z16slice):
            nc.sync.dma_start(
                blk.rearrange("(w p) d -> p w d", p=128), z16slice)
            if collectives:
                nc.gpsimd.collective_compute(
                    "AllGather", mybir.AluOpType.bypass, replica_groups=rg,
                    ins=[blk.opt()], outs=[shr.opt()])
            else:
                nc.sync.dma_start(
                    shr.rearrange("(r n) d -> r n d", r=NCORES)[0], blk[:])

        for l in range(L):
            zt_cur = ztA if l % 2 == 0 else ztB
            zt_next = ztB if l % 2 == 0 else ztA
            srcs = [xA_t.ap(), xB_t.ap()] if l == 0 else \
                   [zshA[l - 1][:], zshB[l - 1][:]]

            icol = 0
            ccol = 0
            for gi, wins in enumerate(GROUPS):
                wg = len(wins)
                nn = wg * 128
                n0 = wins[0] * 128
                pairs = sorted({w // 2 for w in wins})
                cC = [Cl if pr == NPAIR - 1 else C for pr in pairs]
                cum = np.cumsum([0] + cC)
                aggs = [aggp.tile([128, 256], f32, tag="aggw",
                                  name=f"agg_l{l}g{gi}p{pi}")
                        for pi in range(len(pairs))]

                for p in (0, 1):
                    ch0 = 0
                    for nchk in _gather_sizes(int(cum[-1])):
                        gb = gp.tile([128, MAXG // 128, 128], f16, tag="g")
                        nc.gpsimd.dma_gather(
                            gb[:, 0:nchk, :], srcs[p],
                            idxs[:, icol:icol + nchk * 8],
                            nchk * 128, nchk * 128, 128,
                        )
                        mb = mp.tile([128, MAXG // 128, 256], f16, tag="m")
                        dst3 = dsts[:, ccol:ccol + nchk].to_broadcast(
                            (128, nchk, 256))
                        iota3 = iotas[:, 0:nchk * 256].rearrange(
                            "p (c f) -> p c f", f=256)
                        nc.vector.tensor_tensor(
                            mb[:, 0:nchk, :], iota3, dst3,
                            op=mybir.AluOpType.is_equal)
                        for k in range(nchk):
                            fc = ch0 + k
                            pi = int(np.searchsorted(cum, fc, side="right")) - 1
                            c = fc - int(cum[pi])
                            nc.tensor.matmul(
                                aggs[pi][:],
                                lhsT=gb[:, k, :], rhs=mb[:, k, :],
                                start=(p == 0 and c == 0),
                                stop=(p == 1 and c == cC[pi] - 1),
                            )
                        icol += nchk * 8
                        ccol += nchk
                        ch0 += nchk

                if taps and l == 0 and gi == 0:
                    aggt = hp.tile([128, GW * 128], f32, tag="aggtap")
                    for pi in range(len(pairs)):
                        nslots = 256 if pairs[pi] != NPAIR - 1 else 128
                        nc.vector.tensor_copy(
                            aggt[:, pi * 256:pi * 256 + nslots],
                            aggs[pi][:, 0:nslots])
                    nc.sync.dma_start(agg_o.ap(), aggt[:])

                # ---- GIN MLP over this group's nodes ------------------
                ht = hp.tile([128, GW * 128], f16, tag="ht")
                for pi in range(len(pairs)):
                    nslots = 256 if pairs[pi] != NPAIR - 1 else 128
                    nc.vector.tensor_add(
                        ht[:, pi * 256:pi * 256 + nslots],
                        aggs[pi][:, 0:nslots],
                        zt_cur[:, n0 + pi * 256:n0 + pi * 256 + nslots])
                h1 = hp.tile([128, GW * 128], f16, tag="h1")
                for s0 in range(0, nn, 512):
                    s1 = min(s0 + 512, nn)
                    p1 = p1p.tile([128, 512], f32, tag="p1")
                    nc.tensor.matmul(p1[:, 0:s1 - s0],
                                     lhsT=w1s[:, l * D:(l + 1) * D],
                                     rhs=ht[:, s0:s1])
                    nc.scalar.activation(h1[:, s0:s1], p1[:, 0:s1 - s0],
                                         Relu, bias=b1s[:, l:l + 1])
                    p2 = p2p.tile([128, 512], f32, tag="p2")
                    nc.tensor.matmul(p2[:, 0:s1 - s0],
                                     lhsT=w2s[:, l * D:(l + 1) * D],
                                     rhs=h1[:, s0:s1])
                    nc.scalar.activation(zt_next[:, n0 + s0:n0 + s1],
                                         p2[:, 0:s1 - s0],
                                         Relu, bias=b2s[:, l:l + 1])
                if l < L - 1:
                    for wi in range(wg):
                        wa = wins[wi]
                        p2b = p1p.tile([128, 512], f32, tag="p1")
                        nc.tensor.matmul(
                            p2b[:, 0:128],
                            lhsT=h1[:, wi * 128:(wi + 1) * 128],
                            rhs=w2s[:, l * D:(l + 1) * D])
                        zb = zbp.tile([128, 128], f32, tag="zbt")
                        nc.vector.tensor_add(zb[:], p2b[:, 0:128],
                                             b2ms[:, l * D:(l + 1) * D])
                        nc.vector.tensor_scalar_max(z16r[:, wa, :], zb[:], 0.0)

                    if gi == AGA_GROUP:
                        halo(l, zblkA[l], zshA[l], z16r[:, 0:WA, :])

            if l < L - 1:
                halo(l, zblkB[l], zshB[l], z16r[:, WA:W, :])

        nc.sync.dma_start(zout_t.ap(), ztB[:])

    nc.compile()
    return nc


def make_in_maps(inputs, C, Cl, idx_arrs, dst_arrs):
    x = np.asarray(inputs["x"], dtype=np.float32)
    Ws1 = np.asarray(inputs["Ws1"], dtype=np.float32)
    bs1 = np.asarray(inputs["bs1"], dtype=np.float32)
    Ws2 = np.asarray(inputs["Ws2"], dtype=np.float32)
    bs2 = np.asarray(inputs["bs2"], dtype=np.float32)

    x_pad = np.zeros((NPAD, D), np.float32)
    x_pad[:N] = x
    xg16 = x_pad.astype(np.float16).reshape(NCORES, PER_CORE, D)
    xA = np.ascontiguousarray(xg16[:, :NA].reshape(NCORES * NA, D))
    xB = np.ascontiguousarray(xg16[:, NA:].reshape(NCORES * NB, D))
    w1 = np.concatenate([Ws1[l] for l in range(L)], axis=1).astype(np.float16)
    w2 = np.concatenate([Ws2[l] for l in range(L)], axis=1).astype(np.float16)
    b1 = np.ascontiguousarray(bs1.T).astype(np.float32)
    b2 = np.ascontiguousarray(bs2.T).astype(np.float32)
    b2mat = np.concatenate(
        [np.broadcast_to(bs2[l][None, :], (D, D)) for l in range(L)],
        axis=1).astype(np.float32)
    iota = np.broadcast_to(
        np.tile(np.arange(256, dtype=np.float16), MAXG // 128)[None, :],
        (D, (MAXG // 128) * 256)).astype(np.float16)

    in_maps = []
    for c in range(NCORES):
        xT32 = np.ascontiguousarray(
            x_pad[c * PER_CORE:(c + 1) * PER_CORE].T)
        in_maps.append({
            "xA": xA, "xB": xB, "xT32": xT32, "w1": w1, "w2": w2,
            "b1": b1, "b2": b2, "b2mat": b2mat, "iota": iota,
            "idx": idx_arrs[c], "dsts": dst_arrs[c],
        })
    return in_maps


def kernel(x, Ws1, bs1, Ws2, bs2, edge_index):
    C, Cl, idx_arrs, dst_arrs = _prepare_edges(edge_index)
    in_maps = make_in_maps(
        {"x": x, "Ws1": Ws1, "bs1": bs1, "Ws2": Ws2, "bs2": bs2},
        C, Cl, idx_arrs, dst_arrs)

    nc = _build_program(C, Cl)

    from concourse.bass_utils import run_bass_kernel_spmd
    res = run_bass_kernel_spmd(nc, in_maps, core_ids=list(range(NCORES)))
    global last_results
    last_results = res

    out = np.empty((NPAD, D), np.float32)
    for c in range(NCORES):
        out[c * PER_CORE:(c + 1) * PER_CORE] = res.results[c]["zout"].T
    return out[:N]
